# revision 1
# baseline (speedup 1.0000x reference)
"""CLAM-SB MIL forward on 8 Trainium2 NeuronCores (Bass/Tile).

Data-parallel over the bag dimension: core b handles bag b (X[b]: [16384, 1024] f32).
Single pass over X per core:
  - PE transposes X tiles (f32r) -> Xt; h^T = W1^T Xt (PSUM accum over d-chunks)
  - ACT tanh(h + b1) -> th; f columns via th-chunk-stationary matmul vs w2
  - ACT exp(f) -> u grid [128, 128] (col t = row-tile t); w = u * (mask>0)
  - z accumulation: per-tile matmul with w-column stationary, X tile moving
Tail: per-partition top-8 candidates (DVE max8), 64th/65th threshold via
max8/match_replace rounds on a consolidated [2, 1024] row, indirect-DMA gather of
candidate rows, small matmuls vs [Win|Wout], softplus terms, masked sums.
Host combines the per-core scalars into the reference's [10] output.
"""
import numpy as np

import concourse.bacc as bacc
import concourse.bass as bass
import concourse.mybir as mybir
import concourse.tile as tile
from concourse import bass_utils
from concourse.masks import make_identity

f32 = mybir.dt.float32
f32r = mybir.dt.float32r
u32 = mybir.dt.uint32
i32 = mybir.dt.int32
AluOp = mybir.AluOpType
AFT = mybir.ActivationFunctionType
AX = mybir.AxisListType

N, D, A = 16384, 1024, 128
NT = N // 128           # 128 row-tiles
NG = NT // 4            # 32 groups of 4 tiles
NEG = -1.0e30


def build_kernel(stage=99):
    nc = bacc.Bacc("TRN2", target_bir_lowering=False, debug=False, num_devices=8)
    X = nc.dram_tensor("X", [N, D], f32, kind="ExternalInput").ap()
    maskg = nc.dram_tensor("maskg", [128, 128], f32, kind="ExternalInput").ap()
    W1 = nc.dram_tensor("W1", [D, A], f32, kind="ExternalInput").ap()
    b1 = nc.dram_tensor("b1", [128, 1], f32, kind="ExternalInput").ap()
    w2 = nc.dram_tensor("w2", [128, 1], f32, kind="ExternalInput").ap()
    Wd = nc.dram_tensor("Wd", [D, 4], f32, kind="ExternalInput").ap()
    Wc = nc.dram_tensor("Wc", [1, D], f32, kind="ExternalInput").ap()
    cb = nc.dram_tensor("cb", [1, 4], f32, kind="ExternalInput").ap()
    out_vec = nc.dram_tensor("out_vec", [1, 8], f32, kind="ExternalOutput").ap()
    out_cnt = nc.dram_tensor("out_cnt", [2, 2], f32, kind="ExternalOutput").ap()

    with tile.TileContext(nc) as tc:
        consts = tc.alloc_tile_pool(name="consts", bufs=1)
        # identity (f32r) for PE transposes
        ident = consts.tile([128, 128], f32)
        make_identity(nc, ident[:])
        identr = consts.tile([128, 128], f32r)
        nc.vector.tensor_copy(identr[:], ident[:])
        # W1 as [128, 8, 128]: [k, c, a] = W1[128c + k, a]
        W1sb = consts.tile([128, 8, 128], f32)
        nc.sync.dma_start(W1sb[:], W1.rearrange("(c p) a -> p c a", p=128))
        W1r = consts.tile([128, 8, 128], f32r)
        nc.vector.tensor_copy(W1r[:], W1sb[:])
        b1sb = consts.tile([128, 1], f32)
        nc.sync.dma_start(b1sb[:], b1[:])
        w2sb = consts.tile([128, 4], f32)
        nc.vector.memset(w2sb[:], 0.0)
        nc.sync.dma_start(w2sb[:, 0:1], w2[:])
        w2r = consts.tile([128, 4], f32r)
        nc.vector.tensor_copy(w2r[:], w2sb[:])
        Wdsb = consts.tile([128, 8, 4], f32)
        nc.sync.dma_start(Wdsb[:], Wd.rearrange("(c p) k -> p c k", p=128))
        Wdr = consts.tile([128, 8, 4], f32r)
        nc.vector.tensor_copy(Wdr[:], Wdsb[:])
        Wcsb = consts.tile([1, D], f32)
        nc.sync.dma_start(Wcsb[:], Wc[:])
        cbsb = consts.tile([1, 4], f32)
        nc.sync.dma_start(cbsb[:], cb[:])
        masksb = consts.tile([128, 128], f32)
        nc.sync.dma_start(masksb[:], maskg[:])
        mask01 = consts.tile([128, 128], f32)
        nc.vector.tensor_scalar(mask01[:], masksb[:], 0.0, None, op0=AluOp.is_gt)
        iota_p = consts.tile([128, 1], i32)
        nc.gpsimd.iota(iota_p[:], pattern=[[0, 1]], base=0, channel_multiplier=1)
        iota_pf = consts.tile([128, 1], f32)
        nc.vector.tensor_copy(iota_pf[:], iota_p[:])
        onesf = consts.tile([128, 4], f32)
        nc.vector.memset(onesf[:], 1.0)
        onesr = consts.tile([128, 4], f32r)
        nc.vector.tensor_copy(onesr[:], onesf[:])

        # persistent grids
        u_grid = consts.tile([128, 128], f32r)    # exp(f), col t = tile t
        w_grid = consts.tile([128, 128], f32r)    # u * mask01

        # ---- streaming pools (note stack order: z psum first so it outlives others)
        zpool = tc.alloc_tile_pool(name="zpool", bufs=1, space="PSUM")
        z0 = zpool.tile([1, 512], f32)
        z1 = zpool.tile([1, 512], f32)
        xpool = tc.alloc_tile_pool(name="xpool", bufs=12)
        xtgp = tc.alloc_tile_pool(name="xtgp", bufs=3)
        thp = tc.alloc_tile_pool(name="thp", bufs=3)
        ps_xt = tc.alloc_tile_pool(name="ps_xt", bufs=3, space="PSUM")
        ps_h = tc.alloc_tile_pool(name="ps_h", bufs=2, space="PSUM")
        ps_f = tc.alloc_tile_pool(name="ps_f", bufs=1, space="PSUM")

        for g in range(NG):
            xt_g = xtgp.tile([128, 8, 512], f32r, name=f"xtg{g}", tag="xtg")
            xpair = []
            for d2 in range(2):
                x2 = xpool.tile([128, 2, D], f32r, name=f"x{g}_{d2}", tag="x2", bufs=5)
                r0 = 256 * (2 * g + d2)
                nc.gpsimd.dma_start(
                    x2[:], X[r0:r0 + 256, :].rearrange("(a p) d -> p a d", p=128))
                xpair.append(x2)
            xtiles = [xpair[i // 2][:, i % 2] for i in range(4)]
            for t4 in range(4):
                t = 4 * g + t4
                xt_tile = xtiles[t4]
                for h in range(2):
                    ptr = ps_xt.tile([128, 512], f32r, name=f"pxt{t}_{h}", tag="pxt")
                    for i in range(4):
                        c = 4 * h + i
                        nc.tensor.transpose(
                            ptr[:, 128 * i:128 * (i + 1)],
                            xt_tile[:, 128 * c:128 * (c + 1)],
                            identr[:],
                        )
                    # copy psum -> xt_g[:, 4h:4h+4, 128*t4:128*t4+128]
                    dst = xt_g[:, 4 * h:4 * h + 4, 128 * t4:128 * (t4 + 1)]
                    src = ptr.rearrange("p (c q) -> p c q", c=4)
                    if (t + h) % 2 == 0:
                        nc.vector.tensor_copy(dst, src)
                    else:
                        nc.scalar.copy(dst, src)

            # h^T = sum_c W1_c^T Xt_c  -> [a=128, 512 rows]
            ph = ps_h.tile([128, 512], f32, name=f"ph{g}", tag="ph")
            for c in range(8):
                nc.tensor.matmul(ph[:], W1r[:, c, :], xt_g[:, c, :],
                                 start=(c == 0), stop=(c == 7))
            th = thp.tile([128, 512], f32r, name=f"th{g}", tag="th")
            nc.scalar.activation(th[:], ph[:], AFT.Tanh, bias=b1sb[:, :1], scale=1.0)

            # f columns: lhsT = th chunk [K=a, M=128 rows], rhs = w2 -> [128, 1]
            pf = ps_f.tile([128, 16], f32, name=f"pf{g}", tag="pf")
            for t4 in range(4):
                nc.tensor.matmul(pf[:, 4 * t4:4 * t4 + 4],
                                 th[:, 128 * t4:128 * (t4 + 1)], w2r[:],
                                 start=True, stop=True)
            # u = exp(f); w = u * mask01  (f is every 4th column of pf)
            nc.scalar.activation(u_grid[:, 4 * g:4 * g + 4],
                                 pf[:].rearrange("p (t q) -> p t q", q=4)[:, :, 0:1],
                                 AFT.Exp, bias=0.0, scale=1.0)
            nc.vector.tensor_tensor(w_grid[:, 4 * g:4 * g + 4],
                                    u_grid[:, 4 * g:4 * g + 4].bitcast(f32),
                                    mask01[:, 4 * g:4 * g + 4], op=AluOp.mult)

            # z accumulation: per tile, lhsT = w column, rhs = X tile
            for t4 in range(4):
                t = 4 * g + t4
                nc.tensor.matmul(z0[:], w_grid[:, t:t + 1],
                                 xtiles[t4][:, 0:512],
                                 start=(t == 0), stop=(t == NT - 1),
                                 skip_group_check=True)
                nc.tensor.matmul(z1[:], w_grid[:, t:t + 1],
                                 xtiles[t4][:, 512:1024],
                                 start=(t == 0), stop=(t == NT - 1),
                                 skip_group_check=True)

        ps_f.release()
        ps_h.release()
        ps_xt.release()

        # ---------- tail ----------
        tailp = tc.alloc_tile_pool(name="tailp", bufs=1)
        ps_zf = tc.alloc_tile_pool(name="ps_zf", bufs=1, space="PSUM")

        # L = sum(w_grid); z /= L
        Lpart = tailp.tile([128, 1], f32r)
        with nc.allow_low_precision("f32r partial sums feed exact f32 PSUM reduce"):
            nc.vector.tensor_reduce(Lpart[:], w_grid.bitcast(f32)[:], axis=AX.X, op=AluOp.add)
        pL = ps_zf.tile([1, 4], f32)
        nc.tensor.matmul(pL[:], Lpart[:], onesr[:], start=True, stop=True)
        recipL = tailp.tile([1, 1], f32)
        nc.vector.reciprocal(recipL[:], pL[:, 0:1])
        z_sb = tailp.tile([1, D], f32)
        nc.scalar.activation(z_sb[:, 0:512], z0[:], AFT.Copy, bias=0.0, scale=recipL[:, :1])
        nc.scalar.activation(z_sb[:, 512:1024], z1[:], AFT.Copy, bias=0.0, scale=recipL[:, :1])

        if stage < 0:
            nc.sync.dma_start(out_vec[:], z_sb[:, 0:8])
        else:
            outt = tailp.tile([1, 8], f32)
            nc.vector.memset(outt[:], 0.0)
            scr = tailp.tile([1, D], f32)
            nc.vector.tensor_tensor(scr[:], z_sb[:], Wcsb[:], op=AluOp.mult)
            nc.vector.tensor_reduce(outt[:, 0:1], scr[:], axis=AX.X, op=AluOp.add)

            # candidates: top-8 per partition of u (and of -u)
            v8 = tailp.tile([128, 8], f32)
            i8 = tailp.tile([128, 8], u32)
            nc.vector.max(v8[:], u_grid.bitcast(f32)[:])
            nc.vector.max_index(i8[:], v8[:], u_grid.bitcast(f32)[:])
            uneg = tailp.tile([128, 128], f32)
            nc.vector.tensor_scalar(uneg[:], u_grid.bitcast(f32)[:], -1.0, None, op0=AluOp.mult)
            v8b = tailp.tile([128, 8], f32)
            i8b = tailp.tile([128, 8], u32)
            nc.vector.max(v8b[:], uneg[:])
            nc.vector.max_index(i8b[:], v8b[:], uneg[:])

            # global row indices gidx = col_idx * 128 + p
            def to_gidx(i8t, name):
                i8f = tailp.tile([128, 8], f32, name=name + "f")
                nc.vector.tensor_copy(i8f[:], i8t[:])
                gf = tailp.tile([128, 8], f32, name=name + "gf")
                nc.vector.tensor_scalar(gf[:], i8f[:], 128.0, iota_pf[:, :1],
                                        op0=AluOp.mult, op1=AluOp.add)
                gi = tailp.tile([128, 8], u32, name=name + "gi")
                nc.vector.tensor_copy(gi[:], gf[:])
                return gi

            gidx = to_gidx(i8, "gidx_t")
            gidxb = to_gidx(i8b, "gidx_b")

            # consolidate candidate values to [2, 1024] row form (p-major: col = 8p + c)
            cand2 = tailp.tile([2, 1024], f32)
            nc.sync.dma_start(cand2[0:1, :], v8[:])
            nc.sync.dma_start(cand2[1:2, :], v8b[:])
            candB0 = tailp.tile([1, 1024], f32)
            nc.sync.dma_start(candB0[:], v8b[:])

            # threshold: 8 rounds of max8 + match_replace -> 64th; one more max8 -> 65th
            work = tailp.tile([2, 1024], f32)
            nc.vector.tensor_copy(work[:], cand2[:])
            m8 = tailp.tile([2, 8], f32)
            v64 = tailp.tile([2, 1], f32)
            for r in range(8):
                nc.vector.max(m8[:], work[:])
                if r == 7:
                    nc.vector.tensor_copy(v64[:], m8[:, 7:8])
                nc.vector.match_replace(work[:], m8[:], work[:], NEG)
            m8b = tailp.tile([2, 8], f32)
            nc.vector.max(m8b[:], work[:])
            thr2 = tailp.tile([2, 1], f32)
            nc.vector.tensor_scalar(thr2[:], v64[:], m8b[:, 0:1], 0.5,
                                    op0=AluOp.add, op1=AluOp.mult)

            # selection rows + counts (everything on partition 0)
            thrB0 = tailp.tile([1, 1], f32)
            nc.sync.dma_start(thrB0[:], thr2[1:2, :1])
            selT = tailp.tile([1, 1024], f32)
            nc.vector.tensor_scalar(selT[:], cand2[0:1, :], thr2[0:1, :1], None, op0=AluOp.is_gt)
            selB = tailp.tile([1, 1024], f32)
            nc.vector.tensor_scalar(selB[:], candB0[:], thrB0[:, :1], None, op0=AluOp.is_gt)
            cnts = tailp.tile([1, 4], f32)
            nc.vector.tensor_reduce(cnts[:, 0:1], selT[:], axis=AX.X, op=AluOp.add)
            nc.vector.tensor_reduce(cnts[:, 1:2], selB[:], axis=AX.X, op=AluOp.add)
            # 8th-slot hits: p-major layout -> slot c=7 at cols 8p+7 (stride-8 view)
            c8t = tailp.tile([1, 128], f32)
            nc.vector.tensor_copy(c8t[:].rearrange("o (a p) -> o a p", a=1),
                                  selT[:].rearrange("o (p j) -> o j p", p=128)[:, 7:8, :])
            nc.vector.tensor_reduce(cnts[:, 2:3], c8t[:], axis=AX.X, op=AluOp.add)
            c8b = tailp.tile([1, 128], f32)
            nc.vector.tensor_copy(c8b[:].rearrange("o (a p) -> o a p", a=1),
                                  selB[:].rearrange("o (p j) -> o j p", p=128)[:, 7:8, :])
            nc.vector.tensor_reduce(cnts[:, 3:4], c8b[:], axis=AX.X, op=AluOp.add)
            nc.sync.dma_start(out_cnt[:], cnts[:].rearrange("o (a b) -> (o a) b", a=2))

            ps_zf.release()
            zpool.release()

            # gather candidate rows + transpose + arg rows (diff-weight matmuls)

        skip_tail = stage < 1
        if skip_tail and stage >= 0:
            nc.sync.dma_start(out_vec[:], outt[:])
        ps_tail = None
        if not skip_tail:
            ps_tail = tc.alloc_tile_pool(name="ps_tail", bufs=1, space="PSUM")
            arg_ti = ps_tail.tile([1, 1024], f32)   # top, in-class diff (Wd col 0)
            arg_to = ps_tail.tile([1, 1024], f32)   # top, out-class diff (Wd col 2)
            arg_bi = ps_tail.tile([1, 1024], f32)   # bottom, in-class diff (Wd col 1)

            def side_logits(gidx_t, args, side):
                # args: list of (psum_row, wd_col)
                for grp in range(2):
                    xtg_t = xtgp.tile([128, 8, 512], f32r, name=f"xtt{side}{grp}", tag="xtg")
                    for j4 in range(4):
                        j = 4 * grp + j4
                        gt = xpool.tile([128, D], f32r, name=f"g{side}{j}", tag="x", bufs=4)
                        nc.gpsimd.indirect_dma_start(
                            out=gt[:], out_offset=None, in_=X[:],
                            in_offset=bass.IndirectOffsetOnAxis(ap=gidx_t[:, j:j + 1], axis=0))
                        for h in range(2):
                            ptr2 = ps_tail.tile([128, 512], f32r, name=f"pt{side}{j}{h}",
                                                tag="ptail", bufs=2)
                            for i in range(4):
                                c = 4 * h + i
                                nc.tensor.transpose(
                                    ptr2[:, 128 * i:128 * (i + 1)],
                                    gt[:, 128 * c:128 * (c + 1)],
                                    identr[:])
                            dst = xtg_t[:, 4 * h:4 * h + 4, 128 * j4:128 * (j4 + 1)]
                            src = ptr2.rearrange("p (c q) -> p c q", c=4)
                            if (j + h) % 2 == 0:
                                nc.vector.tensor_copy(dst, src)
                            else:
                                nc.scalar.copy(dst, src)
                    for (prow, wcol) in args:
                        for c in range(8):
                            nc.tensor.matmul(prow[:, 512 * grp:512 * (grp + 1)],
                                             Wdr[:, c, wcol:wcol + 1], xtg_t[:, c, :],
                                             start=(c == 0), stop=(c == 7))

            side_logits(gidx, [(arg_ti, 0), (arg_to, 2)], "t")
            side_logits(gidxb, [(arg_bi, 1)], "b")

            # softplus terms and masked sums
            def wsum(argrow, biascol, selr, outslot, name):
                ee = tailp.tile([1, 1024], f32, name=name + "e")
                nc.scalar.activation(ee[:], argrow[:], AFT.Exp,
                                     bias=cbsb[:, biascol:biascol + 1], scale=1.0)
                sp = tailp.tile([1, 1024], f32, name=name + "s")
                nc.scalar.activation(sp[:], ee[:], AFT.Ln, bias=1.0, scale=1.0)
                # sp cols are j-major (128j + p); selr cols are p-major (8p + j):
                # reorder sp to p-major with a strided copy, then flat TTR.
                sp_pm = tailp.tile([1, 1024], f32, name=name + "pm")
                nc.vector.tensor_copy(sp_pm[:].rearrange("o (p j) -> o p j", p=128),
                                      sp[:].rearrange("o (j p) -> o p j", p=128))
                ws = tailp.tile([1, 1024], f32, name=name + "w")
                nc.vector.tensor_tensor(ws[:], sp_pm[:], selr, op=AluOp.mult)
                nc.vector.tensor_reduce(outt[:, outslot:outslot + 1], ws[:],
                                        axis=AX.X, op=AluOp.add)

            wsum(arg_ti, 0, selT[:], 1, "it")   # in-loss, top (y=1)
            wsum(arg_bi, 1, selB[:], 2, "ib")   # in-loss, bottom (y=0)
            wsum(arg_to, 2, selT[:], 3, "ot")   # out-loss, top (y=0)


            nc.sync.dma_start(out_vec[:], outt[:])

        if ps_tail is not None:
            ps_tail.release()
        tailp.release()
        thp.release()
        xtgp.release()
        xpool.release()
        consts.release()

    nc.compile()
    return nc


_NC_CACHE = None


def _get_nc():
    global _NC_CACHE
    if _NC_CACHE is None:
        import os
        _NC_CACHE = build_kernel(int(os.environ.get("KSTAGE", "99")))
    return _NC_CACHE


def make_in_maps(X, mask, labels, W1, b1, w2, b2, Wc, bc, Wi, bi):
    X = np.asarray(X, dtype=np.float32)
    mask = np.asarray(mask, dtype=np.float32)
    labels = np.asarray(labels).astype(np.int64)
    W1 = np.asarray(W1, dtype=np.float32)
    b1v = np.asarray(b1, dtype=np.float32).reshape(128, 1)
    w2v = np.asarray(w2, dtype=np.float32).reshape(128, 1)
    Wc = np.asarray(Wc, dtype=np.float32)
    Wi = np.asarray(Wi, dtype=np.float32)
    bi = np.asarray(bi, dtype=np.float32)
    in_maps = []
    for b in range(8):
        lab = int(labels[b])
        Win, Wout = Wi[lab], Wi[1 - lab]
        Wdm = np.stack([Win[:, 0] - Win[:, 1],
                        Win[:, 1] - Win[:, 0],
                        Wout[:, 1] - Wout[:, 0],
                        np.zeros(D, np.float32)], axis=1)  # [1024, 4]
        bin_, bout = bi[lab], bi[1 - lab]
        cb = np.array([[1.0 + bin_[0] - bin_[1],
                        1.0 + bin_[1] - bin_[0],
                        1.0 + bout[1] - bout[0], 0.0]], dtype=np.float32)
        maskgrid = np.ascontiguousarray(mask[b].reshape(128, 128).T)
        in_maps.append({
            "X": np.ascontiguousarray(X[b]),
            "maskg": maskgrid,
            "W1": W1,
            "b1": b1v,
            "w2": w2v,
            "Wd": np.ascontiguousarray(Wdm),
            "Wc": Wc.reshape(1, D),
            "cb": cb,
        })
    return in_maps


def assemble(results, labels, bc):
    labels = np.asarray(labels).astype(np.float64)
    bag_pred = np.zeros(8, dtype=np.float64)
    inst = 0.0
    for b in range(8):
        ov = results[b]["out_vec"][0].astype(np.float64)
        bag_pred[b] = ov[0] + float(np.asarray(bc).reshape(-1)[0])
        inst += (ov[1] + ov[2]) / 128.0 + ov[3] / 64.0
    crit = np.mean(np.logaddexp(0.0, bag_pred) - bag_pred * labels)
    out = np.concatenate([bag_pred, [crit], [inst]]).astype(np.float32)
    return out


def kernel(X, mask, labels, W1, b1, w2, b2, Wc, bc, Wi, bi):
    nc = _get_nc()
    in_maps = make_in_maps(X, mask, labels, W1, b1, w2, b2, Wc, bc, Wi, bi)
    res = bass_utils.run_bass_kernel_spmd(nc, in_maps, core_ids=list(range(8)))
    return assemble(res.results, labels, bc)



# revision 2
# speedup vs baseline: 241.9641x; 241.9641x over previous
"""CLAM-SB MIL forward on 8 Trainium2 NeuronCores (Bass/Tile) — optimized v3.

Data-parallel over bags: core b handles bag b (X[b]: [16384, 1024] f32).
Per rep (one full bag forward), single pass over X:
  - ACT casts X tiles f32->bf16
  - PE transposes bf16 tiles (1 cyc/row); DVE copies PSUM->SBUF (bf16, 2x)
  - PE: h^T = W1^T Xt (bf16, f32 PSUM accum); ACT tanh -> th (bf16)
  - PE: f columns via th-chunk matmuls vs w2 (bf16); ACT exp -> u_grid (f32)
  - DVE: w = u * mask01 -> w_grid (bf16); PE: z += w^T X (bf16 rhs)
Tail (emitted interleaved into the NEXT rep's stream so it overlaps the
DMA-bound stream): DVE max8 candidates + 8x(max8+match_replace) threshold,
depth-4 indirect row gather per side, args via tensor_tensor_reduce vs
broadcast Wd columns, ACT softplus, gpsimd partition_all_reduce sums.
out_cnt carries (cntT, cntB, guardT, guardB): counts must be 64/64 and the
guards (selected candidates deeper than slot 4) must be 0.
"""
import numpy as np

import concourse.bacc as bacc
import concourse.bass as bass
import concourse.bass_isa as bass_isa
import concourse.mybir as mybir
import concourse.tile as tile
from concourse import bass_utils
from concourse.masks import make_identity

f32 = mybir.dt.float32
bf16 = mybir.dt.bfloat16
u32 = mybir.dt.uint32
i32 = mybir.dt.int32
AluOp = mybir.AluOpType
AFT = mybir.ActivationFunctionType
AX = mybir.AxisListType
RedOp = bass_isa.ReduceOp

N, D, A = 16384, 1024, 128
NT = N // 128           # 128 row-tiles
NG = NT // 4            # 32 groups of 4 tiles
NEG = -1.0e30
DEPTH = 4               # gather depth per partition per side


def build_kernel(stage=99, reps=1):
    nc = bacc.Bacc("TRN2", target_bir_lowering=False, debug=False, num_devices=8)
    X = nc.dram_tensor("X", [N, D], f32, kind="ExternalInput").ap()
    maskg = nc.dram_tensor("maskg", [128, 128], f32, kind="ExternalInput").ap()
    W1 = nc.dram_tensor("W1", [D, A], f32, kind="ExternalInput").ap()
    b1 = nc.dram_tensor("b1", [128, 1], f32, kind="ExternalInput").ap()
    w2 = nc.dram_tensor("w2", [128, 1], f32, kind="ExternalInput").ap()
    WdT = nc.dram_tensor("WdT", [3, D], f32, kind="ExternalInput").ap()
    Wc = nc.dram_tensor("Wc", [1, D], f32, kind="ExternalInput").ap()
    cb = nc.dram_tensor("cb", [1, 4], f32, kind="ExternalInput").ap()
    out_vec = nc.dram_tensor("out_vec", [1, 8], f32, kind="ExternalOutput").ap()
    out_cnt = nc.dram_tensor("out_cnt", [2, 2], f32, kind="ExternalOutput").ap()

    with tile.TileContext(nc) as tc:
        consts = tc.alloc_tile_pool(name="consts", bufs=1)
        identb = consts.tile([128, 128], bf16)
        W1b = consts.tile([128, 8, 128], bf16)
        b1sb = consts.tile([128, 1], f32)
        w2b = consts.tile([128, 4], bf16)
        Wdb = consts.tile([128, 3, D], f32)
        Wcsb = consts.tile([1, D], f32)
        cbv = consts.tile([128, 4], f32)
        mask01 = consts.tile([128, 128], f32)
        iota_pf = consts.tile([128, 1], f32)
        onesf4 = consts.tile([128, 4], f32)

        initp = tc.alloc_tile_pool(name="initp", bufs=1)
        ident = initp.tile([128, 128], f32)
        make_identity(nc, ident[:])
        nc.vector.tensor_copy(identb[:], ident[:])
        W1sb = initp.tile([128, 8, 128], f32)
        nc.sync.dma_start(W1sb[:], W1.rearrange("(c p) a -> p c a", p=128))
        nc.vector.tensor_copy(W1b[:], W1sb[:])
        nc.sync.dma_start(b1sb[:], b1[:])
        w2sb = initp.tile([128, 1], f32)
        nc.sync.dma_start(w2sb[:], w2[:])
        nc.vector.memset(w2b[:], 0.0)
        nc.vector.tensor_copy(w2b[:, 0:1], w2sb[:])
        # Wd columns broadcast to all partitions: Wdb[:, k, :] = WdT[k, :]
        WdTsb = initp.tile([1, 3, D], f32)
        nc.sync.dma_start(WdTsb[:], WdT[:].rearrange("(o k) d -> o k d", o=1))
        for k in range(3):
            nc.gpsimd.partition_broadcast(Wdb[:, k, :], WdTsb[0:1, k, :])
        nc.sync.dma_start(Wcsb[:], Wc[:])
        cbsb = initp.tile([1, 4], f32)
        nc.sync.dma_start(cbsb[:], cb[:])
        nc.gpsimd.partition_broadcast(cbv[:], cbsb[0:1, :])
        masksb = initp.tile([128, 128], f32)
        nc.sync.dma_start(masksb[:], maskg[:])
        nc.vector.tensor_scalar(mask01[:], masksb[:], 0.0, None, op0=AluOp.is_gt)
        iota_p = initp.tile([128, 1], i32)
        nc.gpsimd.iota(iota_p[:], pattern=[[0, 1]], base=0, channel_multiplier=1)
        nc.vector.tensor_copy(iota_pf[:], iota_p[:])
        nc.vector.memset(onesf4[:], 1.0)
        initp.release()

        # per-rep rotating pools
        gridp = tc.alloc_tile_pool(name="gridp", bufs=2)
        zsbp = tc.alloc_tile_pool(name="zsbp", bufs=2)
        xpool = tc.alloc_tile_pool(name="xpool", bufs=6)
        xbp = tc.alloc_tile_pool(name="xbp", bufs=6)
        xtgp = tc.alloc_tile_pool(name="xtgp", bufs=3)
        thp = tc.alloc_tile_pool(name="thp", bufs=3)
        gtp = tc.alloc_tile_pool(name="gtp", bufs=4)
        ttp = tc.alloc_tile_pool(name="ttp", bufs=1)
        tailp = tc.alloc_tile_pool(name="tailp", bufs=1)
        zpool = tc.alloc_tile_pool(name="zpool", bufs=1, space="PSUM")
        z0 = zpool.tile([1, 512], f32)
        z1 = zpool.tile([1, 512], f32)
        ps_xt = tc.alloc_tile_pool(name="ps_xt", bufs=2, space="PSUM")
        ps_h = tc.alloc_tile_pool(name="ps_h", bufs=2, space="PSUM")
        ps_f = tc.alloc_tile_pool(name="ps_f", bufs=1, space="PSUM")
        ps_sm = tc.alloc_tile_pool(name="ps_sm", bufs=1, space="PSUM")

        state = {}

        def emit_stream_group(rep, g, st):
            R = f"r{rep}_"
            xt_g = xtgp.tile([128, 8, 512], bf16, name=f"{R}xtg{g}", tag="xtg")
            xbtiles = []
            for d2 in range(2):
                x2 = xpool.tile([128, 2, D], f32, name=f"{R}x{g}_{d2}", tag="x2")
                r0 = 256 * (2 * g + d2)
                nc.gpsimd.dma_start(
                    x2[:], X[r0:r0 + 256, :].rearrange("(a p) d -> p a d", p=128))
                xb2 = xbp.tile([128, 2, D], bf16, name=f"{R}xb{g}_{d2}", tag="xb2")
                nc.scalar.copy(xb2[:], x2[:])
                xbtiles.append(xb2)
            xts = [xbtiles[i // 2][:, i % 2] for i in range(4)]
            for t4 in range(4):
                t = 4 * g + t4
                for h in range(2):
                    ptr = ps_xt.tile([128, 512], bf16, name=f"{R}pxt{t}_{h}", tag="pxt")
                    for i in range(4):
                        c = 4 * h + i
                        nc.tensor.transpose(
                            ptr[:, 128 * i:128 * (i + 1)],
                            xts[t4][:, 128 * c:128 * (c + 1)],
                            identb[:],
                        )
                    dst = xt_g[:, 4 * h:4 * h + 4, 128 * t4:128 * (t4 + 1)]
                    src = ptr.rearrange("p (c q) -> p c q", c=4)
                    nc.vector.tensor_copy(dst, src)

            ph = ps_h.tile([128, 512], f32, name=f"{R}ph{g}", tag="ph")
            for c in range(8):
                nc.tensor.matmul(ph[:], W1b[:, c, :], xt_g[:, c, :],
                                 start=(c == 0), stop=(c == 7))
            th = thp.tile([128, 512], bf16, name=f"{R}th{g}", tag="th")
            nc.scalar.activation(th[:], ph[:], AFT.Tanh, bias=b1sb[:, :1], scale=1.0)

            pf = ps_f.tile([128, 16], f32, name=f"{R}pf{g}", tag="pf")
            for t4 in range(4):
                nc.tensor.matmul(pf[:, 4 * t4:4 * t4 + 4],
                                 th[:, 128 * t4:128 * (t4 + 1)], w2b[:],
                                 start=True, stop=True)
            u_grid, w_grid = st["u"], st["w"]
            nc.scalar.activation(u_grid[:, 4 * g:4 * g + 4],
                                 pf[:].rearrange("p (t q) -> p t q", q=4)[:, :, 0:1],
                                 AFT.Exp, bias=0.0, scale=1.0)
            nc.vector.tensor_tensor(w_grid[:, 4 * g:4 * g + 4],
                                    u_grid[:, 4 * g:4 * g + 4],
                                    mask01[:, 4 * g:4 * g + 4], op=AluOp.mult)
            for t4 in range(4):
                t = 4 * g + t4
                nc.tensor.matmul(z0[:], w_grid[:, t:t + 1],
                                 xts[t4][:, 0:512],
                                 start=(t == 0), stop=(t == NT - 1),
                                 skip_group_check=True)
                nc.tensor.matmul(z1[:], w_grid[:, t:t + 1],
                                 xts[t4][:, 512:1024],
                                 start=(t == 0), stop=(t == NT - 1),
                                 skip_group_check=True)

        def emit_stream_epilogue(rep, st):
            R = f"r{rep}_"
            w_grid = st["w"]
            sm = ps_sm.tile([128, 12], f32, name=R + "sm", tag="sm")
            st["sm"] = sm
            Lrow = tailp.tile([128, 1], f32, name=R + "Lrow", tag="Lrow")
            nc.vector.tensor_reduce(Lrow[:], w_grid[:], axis=AX.X, op=AluOp.add)
            nc.tensor.matmul(sm[0:1, 0:4], Lrow[:], onesf4[:], start=True, stop=True)
            recipL = tailp.tile([1, 1], f32, name=R + "rL", tag="rL")
            nc.vector.reciprocal(recipL[:], sm[0:1, 0:1])
            z_sb = zsbp.tile([1, D], f32, name=R + "zsb", tag="zsb")
            nc.scalar.activation(z_sb[:, 0:512], z0[:], AFT.Copy, bias=0.0,
                                 scale=recipL[:, :1])
            nc.scalar.activation(z_sb[:, 512:1024], z1[:], AFT.Copy, bias=0.0,
                                 scale=recipL[:, :1])
            st["z_sb"] = z_sb

        # ---- tail pieces (for rep; emitted during rep+1's stream) ----
        def tail_a(rep, st):
            """Candidates + threshold chain (DVE-heavy, serial)."""
            R = f"t{rep}_"
            u_grid = st["u"]
            outt = tailp.tile([1, 8], f32, name=R + "outt", tag="outt")
            nc.vector.memset(outt[:], 0.0)
            st["outt"] = outt
            scrb = tailp.tile([1, D], f32, name=R + "scrb", tag="scrb")
            nc.vector.tensor_tensor(scrb[:], st["z_sb"][:], Wcsb[:], op=AluOp.mult)
            nc.vector.tensor_reduce(outt[:, 0:1], scrb[:], axis=AX.X, op=AluOp.add)

            v8 = tailp.tile([128, 8], f32, name=R + "v8", tag="v8")
            i8 = tailp.tile([128, 8], u32, name=R + "i8", tag="i8")
            nc.vector.max(v8[:], u_grid[:])
            nc.vector.max_index(i8[:], v8[:], u_grid[:])
            uneg = tailp.tile([128, 128], f32, name=R + "uneg", tag="uneg")
            nc.vector.tensor_scalar(uneg[:], u_grid[:], -1.0, None, op0=AluOp.mult)
            v8b = tailp.tile([128, 8], f32, name=R + "v8b", tag="v8b")
            i8b = tailp.tile([128, 8], u32, name=R + "i8b", tag="i8b")
            nc.vector.max(v8b[:], uneg[:])
            nc.vector.max_index(i8b[:], v8b[:], uneg[:])
            st["v8"], st["i8"], st["v8b"], st["i8b"] = v8, i8, v8b, i8b

            cand2 = tailp.tile([2, 1024], f32, name=R + "cand2", tag="cand2")
            nc.sync.dma_start(cand2[0:1, :], v8[:])
            nc.sync.dma_start(cand2[1:2, :], v8b[:])
            st["cand2"] = cand2
            work = tailp.tile([2, 1024], f32, name=R + "work", tag="work")
            nc.vector.tensor_copy(work[:], cand2[:])
            m8 = tailp.tile([2, 8], f32, name=R + "m8", tag="m8")
            v64 = tailp.tile([2, 1], f32, name=R + "v64", tag="v64")
            for r in range(8):
                nc.vector.max(m8[:], work[:])
                if r == 7:
                    nc.vector.tensor_copy(v64[:], m8[:, 7:8])
                nc.vector.match_replace(work[:], m8[:], work[:], NEG)
            m8b = tailp.tile([2, 8], f32, name=R + "m8b", tag="m8b")
            nc.vector.max(m8b[:], work[:])
            thr2 = tailp.tile([2, 1], f32, name=R + "thr2", tag="thr2")
            nc.vector.tensor_scalar(thr2[:], v64[:], m8b[:, 0:1], 0.5,
                                    op0=AluOp.add, op1=AluOp.mult)
            st["thr2"] = thr2

        def tail_b(rep, st):
            """Selection masks, counts, gathers, arg TTRs."""
            R = f"t{rep}_"
            thr2 = st["thr2"]
            # sel in [2, 1024] p-major layout (col = 8p + c); per-partition thr
            sel2 = tailp.tile([2, 1024], f32, name=R + "sel2", tag="sel2")
            nc.vector.tensor_scalar(sel2[:], st["cand2"][:], thr2[:, 0:1], None,
                                    op0=AluOp.is_gt)
            cnt8 = tailp.tile([2, 1], f32, name=R + "cnt8", tag="cnt8")
            nc.vector.tensor_reduce(cnt8[:], sel2[:], axis=AX.X, op=AluOp.add)
            # count of selected candidates within slots c < DEPTH
            sel4 = tailp.tile([2, 512], f32, name=R + "sel4", tag="sel4")
            nc.vector.tensor_copy(sel4[:].rearrange("a (p c) -> a p c", c=DEPTH),
                                  sel2[:].rearrange("a (p c) -> a p c", c=8)[:, :, 0:DEPTH])
            cnt4d = tailp.tile([2, 1], f32, name=R + "cnt4d", tag="cnt4d")
            nc.vector.tensor_reduce(cnt4d[:], sel4[:], axis=AX.X, op=AluOp.add)
            nc.sync.dma_start(out_cnt[:, 0:1], cnt8[:, 0:1])
            nc.sync.dma_start(out_cnt[:, 1:2], cnt4d[:, 0:1])
            # bottom-side sel row moved to partition 0 (DVE can't read p1)
            selBrow = tailp.tile([1, 512], f32, name=R + "selBrow", tag="selBrow")
            nc.sync.dma_start(selBrow[:], sel4[1:2, :])
            st["sel4"], st["selBrow"] = sel4, selBrow

            def to_gidx(i8t, name):
                i8f = tailp.tile([128, DEPTH], f32, name=name + "f", tag=name + "f")
                nc.vector.tensor_copy(i8f[:], i8t[:, 0:DEPTH])
                gf = tailp.tile([128, DEPTH], f32, name=name + "g", tag=name + "g")
                nc.vector.tensor_scalar(gf[:], i8f[:], 128.0, iota_pf[:, :1],
                                        op0=AluOp.mult, op1=AluOp.add)
                gi = tailp.tile([128, DEPTH], u32, name=name + "i", tag=name + "i")
                nc.vector.tensor_copy(gi[:], gf[:])
                return gi

            gidxT = to_gidx(st["i8"], R + "gxt")
            gidxB = to_gidx(st["i8b"], R + "gxb")

            arg_it = tailp.tile([128, DEPTH], f32, name=R + "ait", tag="ait")
            arg_ot = tailp.tile([128, DEPTH], f32, name=R + "aot", tag="aot")
            arg_ib = tailp.tile([128, DEPTH], f32, name=R + "aib", tag="aib")
            st["args"] = (arg_it, arg_ib, arg_ot)
            for c in range(DEPTH):
                gt = gtp.tile([128, D], f32, name=f"{R}gt{c}", tag="gt")
                nc.gpsimd.indirect_dma_start(
                    out=gt[:], out_offset=None, in_=X[:],
                    in_offset=bass.IndirectOffsetOnAxis(ap=gidxT[:, c:c + 1], axis=0))
                scr = ttp.tile([128, D], f32, name=f"{R}sc{c}", tag="scr")
                nc.vector.tensor_tensor(scr[:], gt[:], Wdb[:, 0, :], op=AluOp.mult)
                nc.vector.tensor_reduce(arg_it[:, c:c + 1], scr[:], axis=AX.X, op=AluOp.add)
                scr2 = ttp.tile([128, D], f32, name=f"{R}sd{c}", tag="scr")
                nc.vector.tensor_tensor(scr2[:], gt[:], Wdb[:, 2, :], op=AluOp.mult)
                nc.vector.tensor_reduce(arg_ot[:, c:c + 1], scr2[:], axis=AX.X, op=AluOp.add)
            for c in range(DEPTH):
                gb = gtp.tile([128, D], f32, name=f"{R}gb{c}", tag="gt")
                nc.gpsimd.indirect_dma_start(
                    out=gb[:], out_offset=None, in_=X[:],
                    in_offset=bass.IndirectOffsetOnAxis(ap=gidxB[:, c:c + 1], axis=0))
                scr3 = ttp.tile([128, D], f32, name=f"{R}se{c}", tag="scr")
                nc.vector.tensor_tensor(scr3[:], gb[:], Wdb[:, 1, :], op=AluOp.mult)
                nc.vector.tensor_reduce(arg_ib[:, c:c + 1], scr3[:], axis=AX.X, op=AluOp.add)

        def tail_c(rep, st):
            """Softplus, masked sums (all on partition 0), output DMA."""
            R = f"t{rep}_"
            arg_it, arg_ib, arg_ot = st["args"]
            selTrow = st["sel4"][0:1, :]
            outt = st["outt"]
            for k, (arg, cbcol, selrow) in enumerate(
                    [(arg_it, 0, selTrow), (arg_ib, 1, st["selBrow"][:]),
                     (arg_ot, 2, selTrow)]):
                e = tailp.tile([128, DEPTH], f32, name=f"{R}e{k}", tag=f"e{k}")
                nc.scalar.activation(e[:], arg[:], AFT.Exp,
                                     bias=cbv[:, cbcol:cbcol + 1], scale=1.0)
                sp = tailp.tile([128, DEPTH], f32, name=f"{R}s{k}", tag=f"s{k}")
                nc.scalar.activation(sp[:], e[:], AFT.Ln, bias=1.0, scale=1.0)
                sprow = tailp.tile([1, 512], f32, name=f"{R}r{k}", tag=f"r{k}")
                nc.sync.dma_start(sprow[:], sp[:])
                ws = tailp.tile([1, 512], f32, name=f"{R}w{k}", tag=f"w{k}")
                nc.vector.tensor_tensor(ws[:], sprow[:], selrow, op=AluOp.mult)
                nc.vector.tensor_reduce(outt[:, k + 1:k + 2], ws[:], axis=AX.X,
                                        op=AluOp.add)
            nc.sync.dma_start(out_vec[:], outt[:])

        def emit_stream(rep, prev_st):
            st = {
                "u": gridp.tile([128, 128], f32, name=f"r{rep}_ug", tag="ug"),
                "w": gridp.tile([128, 128], bf16, name=f"r{rep}_wg", tag="wg"),
            }
            for g in range(NG):
                emit_stream_group(rep, g, st)
                if prev_st is not None:
                    if g == 2:
                        tail_a(rep - 1, prev_st)
                    elif g == 8:
                        tail_b(rep - 1, prev_st)
                    elif g == 14:
                        tail_c(rep - 1, prev_st)
            emit_stream_epilogue(rep, st)
            return st

        prev = None
        for rep in range(reps):
            prev = emit_stream(rep, prev)
        tail_a(reps - 1, prev)
        tail_b(reps - 1, prev)
        tail_c(reps - 1, prev)

        ps_sm.release()
        ps_f.release()
        ps_h.release()
        ps_xt.release()
        zpool.release()
        tailp.release()
        ttp.release()
        gtp.release()
        thp.release()
        xtgp.release()
        xbp.release()
        xpool.release()
        zsbp.release()
        gridp.release()
        consts.release()

    nc.compile()
    return nc


_NC_CACHE = None
_EXEC_CACHE = {}


def _get_nc():
    global _NC_CACHE
    if _NC_CACHE is None:
        import os
        _NC_CACHE = build_kernel(int(os.environ.get("KSTAGE", "99")),
                                 reps=int(os.environ.get("KREPS", "1")))
    return _NC_CACHE


def make_fn(nc):
    """Jitted 8-core executor for a compiled Bass module (cached per nc)."""
    import jax
    from concourse.bass2jax import _bass_exec_p, partition_id_tensor
    from jax.sharding import Mesh, PartitionSpec, NamedSharding
    from jax.experimental.shard_map import shard_map

    if id(nc) in _EXEC_CACHE:
        return _EXEC_CACHE[id(nc)]

    partition_name = nc.partition_id_tensor.name if nc.partition_id_tensor else None
    in_names, out_names, out_avals, zero_outs = [], [], [], []
    for alloc in nc.m.functions[0].allocations:
        if not isinstance(alloc, mybir.MemoryLocationSet):
            continue
        name = alloc.memorylocations[0].name
        if alloc.kind == "ExternalInput":
            if name != partition_name:
                in_names.append(name)
        elif alloc.kind == "ExternalOutput":
            out_names.append(name)
            shape = tuple(alloc.tensor_shape)
            dtype = mybir.dt.np(alloc.dtype)
            out_avals.append(jax.core.ShapedArray(shape, dtype))
            zero_outs.append(np.zeros(shape, dtype))
    n_params = len(in_names)
    all_in = list(in_names) + list(out_names)
    if partition_name is not None:
        all_in.append(partition_name)

    def _body(*args):
        operands = list(args)
        if partition_name is not None:
            operands.append(partition_id_tensor())
        outs = _bass_exec_p.bind(
            *operands, out_avals=tuple(out_avals), in_names=tuple(all_in),
            out_names=tuple(out_names), lowering_input_output_aliases=(),
            sim_require_finite=True, sim_require_nnan=True, nc=nc)
        return tuple(outs)

    devices = jax.devices()[:8]
    mesh = Mesh(np.asarray(devices), ("core",))
    n_outs = len(out_names)
    fn = jax.jit(shard_map(_body, mesh=mesh,
                           in_specs=(PartitionSpec("core"),) * (n_params + n_outs),
                           out_specs=(PartitionSpec("core"),) * n_outs,
                           check_rep=False), keep_unused=True)
    sh = NamedSharding(mesh, PartitionSpec("core"))
    entry = (fn, in_names, out_names, out_avals, zero_outs, sh)
    _EXEC_CACHE[id(nc)] = entry
    return entry


def run_spmd(nc, in_maps):
    """Execute nc on 8 cores; returns per-core dict of outputs."""
    import jax
    fn, in_names, out_names, out_avals, zero_outs, sh = make_fn(nc)
    concat_in = [np.concatenate([np.asarray(in_maps[c][n]) for c in range(8)], axis=0)
                 for n in in_names]
    concat_zero = [np.zeros((8 * z.shape[0], *z.shape[1:]), z.dtype) for z in zero_outs]
    args = [jax.device_put(a, sh) for a in concat_in] +            [jax.device_put(z, sh) for z in concat_zero]
    out = fn(*args)
    out_np = [np.asarray(o) for o in out]
    return [
        {name: out_np[i].reshape(8, *out_avals[i].shape)[c]
         for i, name in enumerate(out_names)}
        for c in range(8)
    ]


def make_in_maps(X, mask, labels, W1, b1, w2, b2, Wc, bc, Wi, bi):
    X = np.asarray(X, dtype=np.float32)
    mask = np.asarray(mask, dtype=np.float32)
    labels = np.asarray(labels).astype(np.int64)
    W1 = np.asarray(W1, dtype=np.float32)
    b1v = np.asarray(b1, dtype=np.float32).reshape(128, 1)
    w2v = np.asarray(w2, dtype=np.float32).reshape(128, 1)
    Wc = np.asarray(Wc, dtype=np.float32)
    Wi = np.asarray(Wi, dtype=np.float32)
    bi = np.asarray(bi, dtype=np.float32)
    in_maps = []
    for b in range(8):
        lab = int(labels[b])
        Win, Wout = Wi[lab], Wi[1 - lab]
        WdTm = np.stack([Win[:, 0] - Win[:, 1],
                         Win[:, 1] - Win[:, 0],
                         Wout[:, 1] - Wout[:, 0]], axis=0)  # [3, 1024]
        bin_, bout = bi[lab], bi[1 - lab]
        cbm = np.array([[1.0 + bin_[0] - bin_[1],
                         1.0 + bin_[1] - bin_[0],
                         1.0 + bout[1] - bout[0], 0.0]], dtype=np.float32)
        maskgrid = np.ascontiguousarray(mask[b].reshape(128, 128).T)
        in_maps.append({
            "X": np.ascontiguousarray(X[b]),
            "maskg": maskgrid,
            "W1": W1,
            "b1": b1v,
            "w2": w2v,
            "WdT": np.ascontiguousarray(WdTm),
            "Wc": Wc.reshape(1, D),
            "cb": cbm,
        })
    return in_maps


def assemble(results, labels, bc):
    labels = np.asarray(labels).astype(np.float64)
    bag_pred = np.zeros(8, dtype=np.float64)
    inst = 0.0
    for b in range(8):
        ov = results[b]["out_vec"][0].astype(np.float64)
        bag_pred[b] = ov[0] + float(np.asarray(bc).reshape(-1)[0])
        inst += (ov[1] + ov[2]) / 128.0 + ov[3] / 64.0
    crit = np.mean(np.logaddexp(0.0, bag_pred) - bag_pred * labels)
    out = np.concatenate([bag_pred, [crit], [inst]]).astype(np.float32)
    return out


def kernel(X, mask, labels, W1, b1, w2, b2, Wc, bc, Wi, bi):
    nc = _get_nc()
    in_maps = make_in_maps(X, mask, labels, W1, b1, w2, b2, Wc, bc, Wi, bi)
    results = run_spmd(nc, in_maps)
    return assemble(results, labels, bc)
